# revision 14
# baseline (speedup 1.0000x reference)
"""BigBird sparse attention kernel for Trainium2 (8 NeuronCores, head-parallel).

Strategy (hardcoded for B=1, S=768, D=256, H=8, DH=32, BS=64, M=768):
  - One head per core (B*H = 8 = n_cores). Q/K/V projections computed
    per-core with that head's weight slice; gather indices / masks are
    block-structured and baked into the program as static run-lists.
  - scores output [q, m] is computed by block-sparse matmuls against a
    gathered K^T (built on-chip via SBUF->SBUF DMA run copies); the
    -1e9 pad mask enters through an augmented contraction row.
  - softmax is computed in transposed (dense token) space: S^T + mask
    comes out of one augmented matmul (A-pattern x B-mask rows add the
    block mask inside the PE), exp on ScalarE, context = V^T @ E^T with
    an appended ones-column producing the softmax denominator for free.
  - normalization folds into the context before the output projection;
    per-core partial outputs (head h x Wo rows of head h) are summed on
    the host (standard unshard for a contraction-sharded output) + bias.
"""

import numpy as np

import concourse.bacc as bacc
import concourse.bass as bass
import concourse.tile as tile
from concourse import mybir
from concourse.bass_utils import run_bass_kernel_spmd

# Model dims (fixed for this problem)
B, S, D = 1, 768, 256
H, KD = 8, 256
DH = KD // H            # 32
BS = 64                 # block size
NB = S // BS            # 12 blocks
M = S                   # padded neighbor width for this pattern
NCORES = 8
NT = S // 128           # 6 partition tiles of 128 rows
NEG = -1.0e9
F32 = mybir.dt.float32
F32R = mybir.dt.float32r
EXP = mybir.ActivationFunctionType.Exp

USE_COLPACK = True      # pack q-block pairs into one PSUM tile via col tiling


def _derive_neighbors(attn_indices, blocked_mask):
    """Per-q-block sorted neighbor block lists, from the (replicated) inputs."""
    idx = np.asarray(attn_indices)
    assert idx.shape == (S, M), idx.shape
    msk = np.asarray(blocked_mask).reshape(S, M)
    nbrs = []
    for b in range(NB):
        row = b * BS
        cs = []
        for j in range(M // BS):
            if msk[row, j * BS] == 0.0:
                cs.append(int(idx[row, j * BS]) // BS)
        nbrs.append(tuple(cs))
    return tuple(nbrs)


def _runs(cs):
    """Merge a sorted block list into runs of consecutive blocks."""
    runs = []
    s = p = cs[0]
    for c in cs[1:]:
        if c == p + 1:
            p = c
        else:
            runs.append((s, p))
            s = p = c
    runs.append((s, p))
    return runs


_PROG_CACHE = {}


def _build_program(pattern, with_bias):
    nbrs = [list(t) for t in pattern]
    nc = bacc.Bacc(
        "TRN2", target_bir_lowering=False, debug=False, num_devices=NCORES
    )

    # ---- DRAM parameters (per-core values supplied via in_maps) ----
    d_xq = nc.dram_tensor("xqt", [D, S], F32R, kind="ExternalInput").ap()
    d_xk = nc.dram_tensor("xkt", [D, S], F32R, kind="ExternalInput").ap()
    d_xv = nc.dram_tensor("xvt", [D, S], F32R, kind="ExternalInput").ap()
    d_wq = nc.dram_tensor("wq", [D, DH], F32R, kind="ExternalInput").ap()
    d_wk = nc.dram_tensor("wk", [D, DH], F32R, kind="ExternalInput").ap()
    d_wv = nc.dram_tensor("wv", [D, DH], F32R, kind="ExternalInput").ap()
    d_wo = nc.dram_tensor("wo", [DH, D], F32R, kind="ExternalInput").ap()
    # ob: row 0 = ones, rows 1..12 = B mask (-1e9 where block r not a neighbor
    # of q's block).  az: row 0 = zeros, rows 1..12 = A block-indicator.
    d_ob = nc.dram_tensor("ob", [NB + 1, S], F32R, kind="ExternalInput").ap()
    d_az = nc.dram_tensor("az", [NB + 1, S], F32R, kind="ExternalInput").ap()
    d_pm = nc.dram_tensor("padmask", [1, NB * M], F32R, kind="ExternalInput").ap()
    if with_bias:
        d_bq = nc.dram_tensor("bq", [1, DH], F32R, kind="ExternalInput").ap()
        d_bk = nc.dram_tensor("bk", [1, DH], F32R, kind="ExternalInput").ap()
        d_bv = nc.dram_tensor("bv", [1, DH], F32R, kind="ExternalInput").ap()

    d_scores = nc.dram_tensor("scores", [S, M], F32, kind="ExternalOutput").ap()
    d_outp = nc.dram_tensor("outp", [S, D], F32, kind="ExternalOutput").ap()

    def mm(out, lhsT, rhs, **kw):
        nc.tensor.matmul(out, lhsT, rhs, **kw)

    with tile.TileContext(nc) as tc:
        with (
            tc.tile_pool(name="sb", bufs=1) as sb,
            tc.tile_pool(name="pp", bufs=2, space="PSUM") as pp,
            tc.tile_pool(name="pso", bufs=1, space="PSUM") as pso,
            tc.tile_pool(name="pctx", bufs=2, space="PSUM") as pctx,
            tc.tile_pool(name="prs", bufs=2, space="PSUM") as prs,
        ):
            # ---- persistent SBUF tensors ----
            xq_sb = sb.tile([128, 2, S], F32R)
            xk_sb = sb.tile([128, 2, S], F32R)
            xv_sb = sb.tile([128, 2, S], F32R)
            wq_sb = sb.tile([128, 2, DH], F32R)
            wk_sb = sb.tile([128, 2, DH], F32R)
            wv_sb = sb.tile([128, 2, DH], F32R)
            wo_sb = sb.tile([DH, D], F32R)
            qta = sb.tile([32 + 1 + NB, S], F32R)   # qT | ones | B
            kta = sb.tile([32 + 1 + NB, S], F32R)   # kT | zeros | A
            kg = sb.tile([33, NB * M], F32R)        # gathered K^T | padmask row
            vaug = sb.tile([128, NT, DH], F32R)    # projected V (per t-tile)
            et = sb.tile([128, NT, 2, 384], F32R)   # exp(S^T + mask)
            sout = sb.tile([64, NB, M], F32)       # gathered scores out (per block)
            ctxu = sb.tile([DH, S], F32)
            ones_col = sb.tile([128, 1], F32R)
            ones32 = sb.tile([1, DH], F32R)
            rs_sb = sb.tile([1, S], F32R)
            recipb = sb.tile([DH, S], F32)
            ctxn = sb.tile([DH, S], F32R)
            outsb = sb.tile([128, NT, D], F32)
            if with_bias:
                ones_sb = sb.tile([1, S], F32R)
                bq_sb = sb.tile([1, DH], F32R)
                bk_sb = sb.tile([1, DH], F32R)
                bv_sb = sb.tile([1, DH], F32R)

            # ---- input DMAs ----
            for dst, src in ((xq_sb, d_xq), (xk_sb, d_xk), (xv_sb, d_xv)):
                nc.sync.dma_start(
                    out=dst[:, :, :], in_=src.rearrange("(c p) s -> p c s", p=128)
                )
            for dst, src in ((wq_sb, d_wq), (wk_sb, d_wk), (wv_sb, d_wv)):
                nc.sync.dma_start(
                    out=dst[:, :, :], in_=src.rearrange("(c p) d -> p c d", p=128)
                )
            nc.sync.dma_start(out=wo_sb[:, :], in_=d_wo)
            nc.sync.dma_start(out=qta[32 : 45, :], in_=d_ob)
            nc.sync.dma_start(out=kta[32 : 45, :], in_=d_az)
            nc.sync.dma_start(out=kg[32:33, :], in_=d_pm)
            # ones helpers (from the ones row of ob; memset can't write f32r)
            nc.sync.dma_start(
                out=ones_col[:, :], in_=d_ob[0:1, 0:128].rearrange("o s -> s o")
            )
            nc.sync.dma_start(out=ones32[:, :], in_=d_ob[0:1, 0:DH])
            if with_bias:
                nc.sync.dma_start(out=ones_sb[:, :], in_=d_ob[0:1, :])
                nc.sync.dma_start(out=bq_sb[:, :], in_=d_bq)
                nc.sync.dma_start(out=bk_sb[:, :], in_=d_bk)
                nc.sync.dma_start(out=bv_sb[:, :], in_=d_bv)

            # ---- q/k projections: dst rows 0..31 = W.T @ X^T (+ bias) ----
            projs = [(wq_sb, xq_sb, qta), (wk_sb, xk_sb, kta)]
            if with_bias:
                bias_of = {0: bq_sb, 1: bk_sb}
            for pi, (wsb, xsb, dst) in enumerate(projs):
                for h2 in range(2):
                    ns = slice(h2 * 384, h2 * 384 + 384)
                    ps = pp.tile([32, 384], F32, tag="pp", name=f"psqk{pi}{h2}")
                    last = not with_bias
                    mm(ps[:, :], wsb[:, 0, :], xsb[:, 0, ns], start=True, stop=False)
                    mm(ps[:, :], wsb[:, 1, :], xsb[:, 1, ns], start=False, stop=last)
                    if with_bias:
                        mm(
                            ps[:, :],
                            bias_of[pi][:, :],
                            ones_sb[0:1, ns],
                            start=False,
                            stop=True,
                        )
                    nc.vector.tensor_copy(dst[0:32, ns], ps[:, :])

            # ---- V projection (+ ones column for the softmax denominator) ----
            for i in range(NT):
                ss = slice(i * 128, i * 128 + 128)
                pv = pp.tile([128, DH], F32, tag="pp", name=f"psv{i}")
                mm(pv[:, :], xv_sb[:, 0, ss], wv_sb[:, 0, :], start=True, stop=False)
                mm(
                    pv[:, :],
                    xv_sb[:, 1, ss],
                    wv_sb[:, 1, :],
                    start=False,
                    stop=not with_bias,
                )
                if with_bias:
                    mm(pv[:, :], ones_sb[0:1, ss], bv_sb[:, :], start=False, stop=True)
                nc.vector.tensor_copy(vaug[:, i, 0:DH], pv[:, :])

            # ---- gathered K^T build (SBUF->SBUF run copies + pad zero fill) ----
            for b in range(NB):
                dst0 = 0
                for c0, c1 in _runs(nbrs[b]):
                    w = (c1 - c0 + 1) * BS
                    nc.sync.dma_start(
                        out=kg[0:32, b * M + dst0 : b * M + dst0 + w],
                        in_=kta[0:32, c0 * BS : c0 * BS + w],
                    )
                    dst0 += w
                if dst0 < M:
                    # pad columns: any finite K data works (the -1e9 padmask row
                    # dominates; rel effect ~1e-8) — reuse leading kT columns
                    nc.sync.dma_start(
                        out=kg[0:32, b * M + dst0 : (b + 1) * M],
                        in_=kta[0:32, 0 : M - dst0],
                    )

            # ---- gathered scores out: S_out[q in b, :] = q . kg_b + padmask ----
            # per-block layout on partitions 0..63; the output DMA maps
            # (partition p, block b) -> score row b*64+p.
            scores_view = d_scores.rearrange("(b p) m -> p b m", p=64)
            for b in range(NB):
                ps = pso.tile([64, 2, 512], F32, tag="pso", name=f"psso{b}")
                for h2 in range(2):
                    mm(
                        ps[:, h2, 0:384],
                        qta[0:33, b * 64 : b * 64 + 64],
                        kg[0:33, b * M + h2 * 384 : b * M + h2 * 384 + 384],
                        start=True,
                        stop=True,
                    )
                if b % 2 == 0:
                    nc.scalar.copy(
                        sout[:, b, :].rearrange("p (c f) -> p c f", c=2),
                        ps[:, :, 0:384],
                    )
                else:
                    nc.vector.tensor_copy(
                        sout[:, b, :].rearrange("p (c f) -> p c f", c=2),
                        ps[:, :, 0:384],
                    )
                nc.sync.dma_start(out=scores_view[:, b, :], in_=sout[:, b, :])

            # ---- S^T + mask (augmented matmul), exp on ScalarE ----
            for i in range(NT):
                ts = slice(i * 128, i * 128 + 128)
                for h2 in range(2):
                    ns = slice(h2 * 384, h2 * 384 + 384)
                    st = pp.tile([128, 384], F32, tag="pp", name=f"psst{i}{h2}")
                    mm(st[:, :], kta[0:45, ts], qta[0:45, ns], start=True, stop=True)
                    nc.scalar.activation(et[:, i, h2, :], st[:, :], EXP)

            # ---- context + softmax denominator, accumulated over t ----
            ctx_ps = [
                pctx.tile([DH, 384], F32, tag="pctx", name=f"psctx{h2}")
                for h2 in range(2)
            ]
            rs_ps = [
                prs.tile([1, 384], F32, tag="prs", name=f"psrs{h2}")
                for h2 in range(2)
            ]
            for i in range(NT):
                for h2 in range(2):
                    mm(
                        ctx_ps[h2][:, :],
                        vaug[:, i, :],
                        et[:, i, h2, :],
                        start=(i == 0),
                        stop=(i == NT - 1),
                    )
                    mm(
                        rs_ps[h2][:, :],
                        ones_col[:, :],
                        et[:, i, h2, :],
                        start=(i == 0),
                        stop=(i == NT - 1),
                    )
            for h2 in range(2):
                ns = slice(h2 * 384, h2 * 384 + 384)
                nc.scalar.copy(ctxu[:, ns], ctx_ps[h2][:, :])
                nc.scalar.copy(rs_sb[0:1, ns], rs_ps[h2][:, :])

            # ---- normalize: ctx / rowsum, in transposed [dh, s] space ----
            # broadcast the rowsum row across DH partitions with a K=1 matmul
            for h2 in range(2):
                ns = slice(h2 * 384, h2 * 384 + 384)
                rb = pp.tile([DH, 384], F32, tag="pp", name=f"psrb{h2}")
                mm(rb[:, :], ones32[:, :], rs_sb[0:1, ns], start=True, stop=True)
                nc.vector.reciprocal(recipb[:, ns], rb[:, :])
            nc.vector.tensor_mul(ctxn[:, :], ctxu[:, :], recipb[:, :])

            # ---- output projection (partial: this head's slice of Wo) ----
            for i in range(NT):
                ss = slice(i * 128, i * 128 + 128)
                po = pp.tile([128, D], F32, tag="pp", name=f"pso{i}")
                mm(po[:, :], ctxn[:, ss], wo_sb[:, :], start=True, stop=True)
                nc.vector.tensor_copy(outsb[:, i, :], po[:, :])
            nc.sync.dma_start(
                out=d_outp.rearrange("(i p) d -> p i d", p=128), in_=outsb[:, :, :]
            )

    nc.compile()
    return nc


def _host_prep(pattern):
    """Replicated constant tensors derived from the sparsity pattern."""
    nbrs = [list(t) for t in pattern]
    ones = np.ones((1, S), np.float32)
    zeros = np.zeros((1, S), np.float32)
    Bm = np.full((NB, S), NEG, np.float32)
    for q in range(S):
        for r in nbrs[q // BS]:
            Bm[r, q] = 0.0
    Ap = np.zeros((NB, S), np.float32)
    for t in range(S):
        Ap[t // BS, t] = 1.0
    ob = np.concatenate([ones, Bm], axis=0)
    az = np.concatenate([zeros, Ap], axis=0)
    pm = np.zeros((1, NB * M), np.float32)
    for b in range(NB):
        n = len(nbrs[b]) * BS
        pm[0, b * M + n : (b + 1) * M] = NEG
    return ob, az, pm


_last_results = None  # stash for test harness introspection
_last_in_maps = None
_last_nc = None


def kernel(
    query,
    value,
    key_in,
    Wq,
    bq,
    Wk,
    bk,
    Wv,
    bv,
    Wo,
    bo,
    attn_indices,
    blocked_mask,
):
    query = np.asarray(query, np.float32)
    value = np.asarray(value, np.float32)
    key_in = np.asarray(key_in, np.float32)
    Wq = np.asarray(Wq, np.float32)
    Wk = np.asarray(Wk, np.float32)
    Wv = np.asarray(Wv, np.float32)
    Wo = np.asarray(Wo, np.float32)
    bq = np.asarray(bq, np.float32)
    bk = np.asarray(bk, np.float32)
    bv = np.asarray(bv, np.float32)
    bo = np.asarray(bo, np.float32)
    attn_indices = np.asarray(attn_indices, np.int32)
    blocked_mask_np = np.asarray(blocked_mask, np.float32)

    pattern = _derive_neighbors(attn_indices, blocked_mask_np)
    with_bias = bool(np.any(bq) or np.any(bk) or np.any(bv))
    key = (pattern, with_bias)
    if key not in _PROG_CACHE:
        _PROG_CACHE[key] = _build_program(pattern, with_bias)
    nc = _PROG_CACHE[key]

    ob, az, pm = _host_prep(pattern)
    c = float(DH) ** -0.5
    xqt = np.ascontiguousarray(query.reshape(S, D).T)
    xkt = np.ascontiguousarray(key_in.reshape(S, D).T)
    xvt = np.ascontiguousarray(value.reshape(S, D).T)

    in_maps = []
    for h in range(NCORES):
        hs = slice(h * DH, (h + 1) * DH)
        im = {
            "xqt": xqt,
            "xkt": xkt,
            "xvt": xvt,
            "wq": np.ascontiguousarray(Wq[:, hs] * c),
            "wk": np.ascontiguousarray(Wk[:, hs]),
            "wv": np.ascontiguousarray(Wv[:, hs]),
            "wo": np.ascontiguousarray(Wo[hs, :]),
            "ob": ob,
            "az": az,
            "padmask": pm,
        }
        if with_bias:
            im["bq"] = np.ascontiguousarray(bq[hs] * c).reshape(1, DH)
            im["bk"] = np.ascontiguousarray(bk[hs]).reshape(1, DH)
            im["bv"] = np.ascontiguousarray(bv[hs]).reshape(1, DH)
        in_maps.append(im)

    global _last_results, _last_in_maps, _last_nc
    _last_in_maps = in_maps
    _last_nc = nc
    res = run_bass_kernel_spmd(nc, in_maps, list(range(NCORES)))
    _last_results = res

    scores = np.stack([res.results[h]["scores"] for h in range(NCORES)])[None]
    out = np.zeros((S, D), np.float32)
    for h in range(NCORES):
        out += res.results[h]["outp"]
    out = (out + bo[None, :]).reshape(B, S, D)
    return out, scores, attn_indices


# revision 17
# speedup vs baseline: 1.2554x; 1.2554x over previous
"""BigBird sparse attention kernel for Trainium2 (8 NeuronCores, head-parallel).

Strategy (hardcoded for B=1, S=768, D=256, H=8, DH=32, BS=64, M=768):
  - One head per core (B*H = 8 = n_cores). Q/K/V projections computed
    per-core with that head's weight slice; gather indices / masks are
    block-structured and baked into the program as static run-lists.
  - scores output [q, m]: per q-block matmuls whose rhs slices kT runs
    directly (the gather is runs of consecutive blocks); pad columns are
    -1e9 memsets in SBUF (the reference pad value q.k0 - 1e9 rounds to
    -1e9 exactly in fp32).
  - softmax runs in transposed (dense token) space: S^T + block mask
    comes out of one augmented matmul (A-pattern x B-mask rows add the
    mask inside the PE), exp on ScalarE; context = V^T @ E^T with a
    ones column (from the V bias matmul) producing the softmax
    denominator for free.
  - the denominator row is PE-transposed into per-partition scalars and
    folded into the PSUM->SBUF copy of the output projection; per-core
    partial outputs (head h x Wo rows of head h) are summed on the host
    (standard unshard for a contraction-sharded output) + bias.
"""

import numpy as np

import concourse.bacc as bacc
import concourse.tile as tile
from concourse import mybir
from concourse.bass_utils import run_bass_kernel_spmd

# Model dims (fixed for this problem)
B, S, D = 1, 768, 256
H, KD = 8, 256
DH = KD // H            # 32
BS = 64                 # block size
NB = S // BS            # 12 blocks
M = S                   # padded neighbor width for this pattern
NCORES = 8
NT = S // 128           # 6 partition tiles of 128 rows
NEG = -1.0e9
F32 = mybir.dt.float32
F32R = mybir.dt.float32r
EXP = mybir.ActivationFunctionType.Exp


def _derive_neighbors(attn_indices, blocked_mask):
    """Per-q-block sorted neighbor block lists, from the (replicated) inputs."""
    idx = np.asarray(attn_indices)
    assert idx.shape == (S, M), idx.shape
    msk = np.asarray(blocked_mask).reshape(S, M)
    nbrs = []
    for b in range(NB):
        row = b * BS
        cs = []
        for j in range(M // BS):
            if msk[row, j * BS] == 0.0:
                cs.append(int(idx[row, j * BS]) // BS)
        nbrs.append(tuple(cs))
    return tuple(nbrs)


def _runs(cs):
    """Merge a sorted block list into runs of consecutive blocks."""
    runs = []
    s = p = cs[0]
    for c in cs[1:]:
        if c == p + 1:
            p = c
        else:
            runs.append((s, p))
            s = p = c
    runs.append((s, p))
    return runs


def _pieces(cs):
    """(dst_half, off_in_half, width, src_col) matmul pieces for one block:
    runs of consecutive neighbor blocks, split at the 384-col PSUM half."""
    out = []
    d0 = 0
    for c0, c1 in _runs(cs):
        w = (c1 - c0 + 1) * BS
        src = c0 * BS
        while w > 0:
            half, off = divmod(d0, 384)
            take = min(w, 384 - off)
            out.append((half, off, take, src))
            d0 += take
            src += take
            w -= take
    return out, d0


_PROG_CACHE = {}


def _build_program(pattern, with_bias):
    nbrs = [list(t) for t in pattern]
    nc = bacc.Bacc(
        "TRN2", target_bir_lowering=False, debug=False, num_devices=NCORES
    )

    # ---- DRAM parameters (per-core values supplied via in_maps) ----
    d_xq = nc.dram_tensor("xqt", [D, S], F32R, kind="ExternalInput").ap()
    d_xk = nc.dram_tensor("xkt", [D, S], F32R, kind="ExternalInput").ap()
    d_xv = nc.dram_tensor("xvt", [D, S], F32R, kind="ExternalInput").ap()
    d_wq = nc.dram_tensor("wq", [D, DH], F32R, kind="ExternalInput").ap()
    d_wk = nc.dram_tensor("wk", [D, DH], F32R, kind="ExternalInput").ap()
    d_wv = nc.dram_tensor("wv", [D, DH + 2], F32R, kind="ExternalInput").ap()
    d_bva = nc.dram_tensor("bva", [1, DH + 2], F32R, kind="ExternalInput").ap()
    d_wo = nc.dram_tensor("wo", [DH, D], F32R, kind="ExternalInput").ap()
    # ob: row 0 = ones, rows 1..12 = B mask (-1e9 where block r is not a
    # neighbor of q's block).  az: rows 0..11 = A block-indicator.
    d_ob = nc.dram_tensor("ob", [NB + 1, S], F32R, kind="ExternalInput").ap()
    d_az = nc.dram_tensor("az", [NB, S], F32R, kind="ExternalInput").ap()
    if with_bias:
        d_bq = nc.dram_tensor("bq", [1, DH], F32R, kind="ExternalInput").ap()
        d_bk = nc.dram_tensor("bk", [1, DH], F32R, kind="ExternalInput").ap()

    d_scores = nc.dram_tensor("scores", [S, M], F32, kind="ExternalOutput").ap()
    d_outp = nc.dram_tensor("outp", [S, D], F32, kind="ExternalOutput").ap()

    mm = nc.tensor.matmul

    with tile.TileContext(nc) as tc:
        with (
            tc.tile_pool(name="sb", bufs=1) as sb,
            tc.tile_pool(name="pp", bufs=4, space="PSUM") as pp,
            tc.tile_pool(name="pso", bufs=1, space="PSUM") as pso,
            tc.tile_pool(name="pctx", bufs=2, space="PSUM") as pctx,
        ):
            # ---- persistent SBUF tensors ----
            xq_sb = sb.tile([128, 2, S], F32R)
            xk_sb = sb.tile([128, 2, S], F32R)
            xv_sb = sb.tile([128, 2, S], F32R)
            wq_sb = sb.tile([128, 2, DH], F32R)
            wk_sb = sb.tile([128, 2, DH], F32R)
            wv_sb = sb.tile([128, 2, DH + 2], F32R)
            bva_sb = sb.tile([1, DH + 2], F32R)
            wo_sb = sb.tile([DH, D], F32R)
            ones_row = sb.tile([1, S], F32R)
            qta = sb.tile([32 + NB, S], F32R)      # qT | B
            kta = sb.tile([32 + NB, S], F32R)      # kT | A
            vaug = sb.tile([128, NT, DH + 2], F32R)  # V | ones column
            et = sb.tile([128, NT, 2, 384], F32R)   # exp(S^T + mask)
            sout = sb.tile([64, NB, M], F32)       # gathered scores (per block)
            ctxu = sb.tile([DH + 1, S], F32R)      # ctx | rowsum row
            rs_sb = sb.tile([1, S], F32R)
            recip_col = sb.tile([128, 2 * NT], F32)
            outsb = sb.tile([128, NT, D], F32)
            if with_bias:
                bq_sb = sb.tile([1, DH], F32R)
                bk_sb = sb.tile([1, DH], F32R)

            # ---- input DMAs (weights first so matmuls can start early) ----
            nc.sync.dma_start(
                out=wq_sb[:, :, :], in_=d_wq.rearrange("(c p) d -> p c d", p=128)
            )
            for c in range(2):
                nc.sync.dma_start(
                    out=xq_sb[:, c, :], in_=d_xq[c * 128 : c * 128 + 128, :]
                )
            nc.sync.dma_start(
                out=wk_sb[:, :, :], in_=d_wk.rearrange("(c p) d -> p c d", p=128)
            )
            for c in range(2):
                nc.sync.dma_start(
                    out=xk_sb[:, c, :], in_=d_xk[c * 128 : c * 128 + 128, :]
                )
            nc.sync.dma_start(
                out=wv_sb[:, :, :], in_=d_wv.rearrange("(c p) d -> p c d", p=128)
            )
            for c in range(2):
                nc.sync.dma_start(
                    out=xv_sb[:, c, :], in_=d_xv[c * 128 : c * 128 + 128, :]
                )
            nc.sync.dma_start(out=ones_row[:, :], in_=d_ob[0:1, :])
            nc.sync.dma_start(out=bva_sb[:, :], in_=d_bva)
            nc.sync.dma_start(out=qta[32 : 32 + NB, :], in_=d_ob[1 : NB + 1, :])
            nc.sync.dma_start(out=kta[32 : 32 + NB, :], in_=d_az)
            nc.sync.dma_start(out=wo_sb[:, :], in_=d_wo)
            if with_bias:
                nc.sync.dma_start(out=bq_sb[:, :], in_=d_bq)
                nc.sync.dma_start(out=bk_sb[:, :], in_=d_bk)

            # ---- q/k projections: dst rows 0..31 = W.T @ X^T (+ bias) ----
            for pi, (wsb, xsb, dst) in enumerate(
                ((wq_sb, xq_sb, qta), (wk_sb, xk_sb, kta))
            ):
                for h2 in range(2):
                    ns = slice(h2 * 384, h2 * 384 + 384)
                    ps = pp.tile([32, 384], F32, tag="pp", name=f"psqk{pi}{h2}")
                    mm(ps[:, :], wsb[:, 0, :], xsb[:, 0, ns], start=True, stop=False)
                    mm(
                        ps[:, :],
                        wsb[:, 1, :],
                        xsb[:, 1, ns],
                        start=False,
                        stop=not with_bias,
                    )
                    if with_bias:
                        mm(
                            ps[:, :],
                            (bq_sb if pi == 0 else bk_sb)[:, :],
                            ones_row[0:1, ns],
                            start=False,
                            stop=True,
                        )
                    if pi == 0:
                        nc.vector.tensor_copy(dst[0:32, ns], ps[:, :])
                    else:
                        nc.scalar.copy(dst[0:32, ns], ps[:, :])

            # ---- V projection (+ ones column via the bias rank-1 matmul) ----
            for i in range(NT):
                ss = slice(i * 128, i * 128 + 128)
                pv = pp.tile([128, DH + 2], F32, tag="pp", name=f"psv{i}")
                mm(pv[:, :], xv_sb[:, 0, ss], wv_sb[:, 0, :], start=True, stop=False)
                mm(pv[:, :], xv_sb[:, 1, ss], wv_sb[:, 1, :], start=False, stop=False)
                mm(pv[:, :], ones_row[0:1, ss], bva_sb[:, :], start=False, stop=True)
                nc.vector.tensor_copy(vaug[:, i, :], pv[:, :])

            # ---- main interleaved loop over the 6 row-tiles ----
            ctx_ps = [
                pctx.tile([DH + 1, 384], F32, tag="pctx", name=f"psctx{h2}")
                for h2 in range(2)
            ]
            scores_view = d_scores.rearrange("(b p) m -> p b m", p=64)
            for i in range(NT):
                ts = slice(i * 128, i * 128 + 128)
                # S^T + mask (augmented contraction: rows 32.. add A x B)
                for h2 in range(2):
                    ns = slice(h2 * 384, h2 * 384 + 384)
                    st = pp.tile([128, 384], F32, tag="pp", name=f"psst{i}{h2}")
                    mm(
                        st[:, :],
                        kta[0 : 32 + NB, ts],
                        qta[0 : 32 + NB, ns],
                        start=True,
                        stop=True,
                    )
                    nc.scalar.activation(et[:, i, h2, :], st[:, :], EXP)
                # context + rowsum accumulation for this tile
                for h2 in range(2):
                    mm(
                        ctx_ps[h2][:, :],
                        vaug[:, i, 0 : DH + 1],
                        et[:, i, h2, :],
                        start=(i == 0),
                        stop=(i == NT - 1),
                    )
                # gathered scores for q-blocks 2i, 2i+1
                for b in (2 * i, 2 * i + 1):
                    ps = pso.tile([64, 2, 512], F32, tag="pso", name=f"psso{b}")
                    pieces, n64 = _pieces(nbrs[b])
                    for half, off, w, src in pieces:
                        mm(
                            ps[:, half, off : off + w],
                            qta[0:32, b * 64 : b * 64 + 64],
                            kta[0:32, src : src + w],
                            start=True,
                            stop=True,
                        )
                    w0 = min(n64, 384)
                    cp = nc.scalar.copy if b % 2 == 0 else nc.vector.tensor_copy
                    cp(sout[:, b, 0:w0], ps[:, 0, 0:w0])
                    if n64 > 384:
                        cp(sout[:, b, 384:n64], ps[:, 1, 0 : n64 - 384])
                    if n64 < M:
                        nc.vector.memset(sout[:, b, n64:M], NEG)
                    nc.gpsimd.dma_start(out=scores_view[:, b, :], in_=sout[:, b, :])

            # ---- rowsum -> per-partition scalars; normalize in out-proj ----
            for h2 in range(2):
                ns = slice(h2 * 384, h2 * 384 + 384)
                nc.scalar.copy(ctxu[:, ns], ctx_ps[h2][:, :])
                nc.scalar.copy(rs_sb[0:1, ns], ctx_ps[h2][DH : DH + 1, :])
            rs_col = pp.tile([128, 2 * NT], F32, tag="pp", name="psrscol")
            for i in range(NT):
                mm(
                    rs_col[:, 2 * i : 2 * i + 2],
                    rs_sb[0:1, i * 128 : i * 128 + 128],
                    ones_row[0:1, 0:2],
                    start=True,
                    stop=True,
                )
            nc.vector.reciprocal(recip_col[:, :], rs_col[:, :])

            # ---- output projection (partial: this head's slice of Wo) ----
            for i in range(NT):
                po = pp.tile([128, D], F32, tag="pp", name=f"pso{i}")
                mm(
                    po[:, :],
                    ctxu[0:DH, i * 128 : i * 128 + 128],
                    wo_sb[:, :],
                    start=True,
                    stop=True,
                )
                nc.vector.tensor_scalar_mul(
                    outsb[:, i, :], po[:, :], recip_col[:, 2 * i : 2 * i + 1]
                )
            nc.sync.dma_start(
                out=d_outp.rearrange("(i p) d -> p i d", p=128), in_=outsb[:, :, :]
            )

    nc.compile()
    return nc


def _host_prep(pattern):
    """Replicated constant tensors derived from the sparsity pattern."""
    nbrs = [list(t) for t in pattern]
    ones = np.ones((1, S), np.float32)
    Bm = np.full((NB, S), NEG, np.float32)
    for q in range(S):
        for r in nbrs[q // BS]:
            Bm[r, q] = 0.0
    Ap = np.zeros((NB, S), np.float32)
    for t in range(S):
        Ap[t // BS, t] = 1.0
    ob = np.concatenate([ones, Bm], axis=0)
    return ob, Ap


_last_results = None  # stash for test harness introspection
_last_in_maps = None
_last_nc = None


def kernel(
    query,
    value,
    key_in,
    Wq,
    bq,
    Wk,
    bk,
    Wv,
    bv,
    Wo,
    bo,
    attn_indices,
    blocked_mask,
):
    query = np.asarray(query, np.float32)
    value = np.asarray(value, np.float32)
    key_in = np.asarray(key_in, np.float32)
    Wq = np.asarray(Wq, np.float32)
    Wk = np.asarray(Wk, np.float32)
    Wv = np.asarray(Wv, np.float32)
    Wo = np.asarray(Wo, np.float32)
    bq = np.asarray(bq, np.float32)
    bk = np.asarray(bk, np.float32)
    bv = np.asarray(bv, np.float32)
    bo = np.asarray(bo, np.float32)
    attn_indices = np.asarray(attn_indices, np.int32)
    blocked_mask_np = np.asarray(blocked_mask, np.float32)

    pattern = _derive_neighbors(attn_indices, blocked_mask_np)
    with_bias = bool(np.any(bq) or np.any(bk))
    key = (pattern, with_bias)
    if key not in _PROG_CACHE:
        _PROG_CACHE[key] = _build_program(pattern, with_bias)
    nc = _PROG_CACHE[key]

    ob, Ap = _host_prep(pattern)
    c = float(DH) ** -0.5
    xqt = np.ascontiguousarray(query.reshape(S, D).T)
    xkt = np.ascontiguousarray(key_in.reshape(S, D).T)
    xvt = np.ascontiguousarray(value.reshape(S, D).T)

    in_maps = []
    for h in range(NCORES):
        hs = slice(h * DH, (h + 1) * DH)
        wva = np.concatenate(
            [Wv[:, hs], np.zeros((D, 2), np.float32)], axis=1
        )  # col 32: ones column comes from the bias row below; col 33: pad
        bva = (
            np.concatenate([bv[hs], [1.0, 0.0]]).astype(np.float32).reshape(1, DH + 2)
        )
        im = {
            "xqt": xqt,
            "xkt": xkt,
            "xvt": xvt,
            "wq": np.ascontiguousarray(Wq[:, hs] * c),
            "wk": np.ascontiguousarray(Wk[:, hs]),
            "wv": np.ascontiguousarray(wva),
            "bva": bva,
            "wo": np.ascontiguousarray(Wo[hs, :]),
            "ob": ob,
            "az": Ap,
        }
        if with_bias:
            im["bq"] = np.ascontiguousarray(bq[hs] * c).reshape(1, DH)
            im["bk"] = np.ascontiguousarray(bk[hs]).reshape(1, DH)
        in_maps.append(im)

    global _last_results, _last_in_maps, _last_nc
    _last_in_maps = in_maps
    _last_nc = nc
    res = run_bass_kernel_spmd(nc, in_maps, list(range(NCORES)))
    _last_results = res

    scores = np.stack([res.results[h]["scores"] for h in range(NCORES)])[None]
    out = np.zeros((S, D), np.float32)
    for h in range(NCORES):
        out += res.results[h]["outp"]
    out = (out + bo[None, :]).reshape(B, S, D)
    return out, scores, attn_indices


# revision 18
# speedup vs baseline: 1.4416x; 1.1483x over previous
"""BigBird sparse attention kernel for Trainium2 (8 NeuronCores, head-parallel).

Strategy (hardcoded for B=1, S=768, D=256, H=8, DH=32, BS=64, M=768):
  - One head per core (B*H = 8 = n_cores). Q/K/V projections computed
    per-core with that head's weight slice; gather indices / masks are
    block-structured and baked into the program as static run-lists.
  - scores output [q, m]: per q-block matmuls whose rhs slices kT runs
    directly (the gather is runs of consecutive blocks); pad columns are
    -1e9 memsets in SBUF (the reference pad value q.k0 - 1e9 rounds to
    -1e9 exactly in fp32).
  - softmax runs in transposed (dense token) space: S^T + block mask
    comes out of one augmented matmul (A-pattern x B-mask rows add the
    mask inside the PE), exp on ScalarE; context = V^T @ E^T with a
    ones column (from the V bias matmul) producing the softmax
    denominator for free.
  - matmul inputs are bf16 (fp32 PSUM accumulation); the output
    projection and denominator transpose stay fp32r for precision.
  - the denominator row is PE-transposed into per-partition scalars and
    folded into the PSUM->SBUF copy of the output projection; per-core
    partial outputs (head h x Wo rows of head h) are summed on the host
    (standard unshard for a contraction-sharded output) + bias.
"""

import ml_dtypes
import numpy as np

import concourse.bacc as bacc
import concourse.tile as tile
from concourse import mybir
from concourse.bass_utils import run_bass_kernel_spmd

# Model dims (fixed for this problem)
B, S, D = 1, 768, 256
H, KD = 8, 256
DH = KD // H            # 32
BS = 64                 # block size
NB = S // BS            # 12 blocks
M = S                   # padded neighbor width for this pattern
NCORES = 8
NT = S // 128           # 6 partition tiles of 128 rows
NEG = -1.0e9
F32 = mybir.dt.float32
F32R = mybir.dt.float32r
BF16 = mybir.dt.bfloat16
NPBF = ml_dtypes.bfloat16
EXP = mybir.ActivationFunctionType.Exp


def _derive_neighbors(attn_indices, blocked_mask):
    """Per-q-block sorted neighbor block lists, from the (replicated) inputs."""
    idx = np.asarray(attn_indices)
    assert idx.shape == (S, M), idx.shape
    msk = np.asarray(blocked_mask).reshape(S, M)
    nbrs = []
    for b in range(NB):
        row = b * BS
        cs = []
        for j in range(M // BS):
            if msk[row, j * BS] == 0.0:
                cs.append(int(idx[row, j * BS]) // BS)
        nbrs.append(tuple(cs))
    return tuple(nbrs)


def _runs(cs):
    """Merge a sorted block list into runs of consecutive blocks."""
    runs = []
    s = p = cs[0]
    for c in cs[1:]:
        if c == p + 1:
            p = c
        else:
            runs.append((s, p))
            s = p = c
    runs.append((s, p))
    return runs


def _pieces(cs):
    """(dst_half, off_in_half, width, src_col) matmul pieces for one block:
    runs of consecutive neighbor blocks, split at the 384-col PSUM half."""
    out = []
    d0 = 0
    for c0, c1 in _runs(cs):
        w = (c1 - c0 + 1) * BS
        src = c0 * BS
        while w > 0:
            half, off = divmod(d0, 384)
            take = min(w, 384 - off)
            out.append((half, off, take, src))
            d0 += take
            src += take
            w -= take
    return out, d0


_PROG_CACHE = {}


def _build_program(pattern, with_bias):
    nbrs = [list(t) for t in pattern]
    nc = bacc.Bacc(
        "TRN2", target_bir_lowering=False, debug=False, num_devices=NCORES
    )

    # ---- DRAM parameters (per-core values supplied via in_maps) ----
    d_xq = nc.dram_tensor("xqt", [D, S], BF16, kind="ExternalInput").ap()
    d_xk = nc.dram_tensor("xkt", [D, S], BF16, kind="ExternalInput").ap()
    d_xv = nc.dram_tensor("xvt", [D, S], BF16, kind="ExternalInput").ap()
    d_wq = nc.dram_tensor("wq", [D, DH], BF16, kind="ExternalInput").ap()
    d_wk = nc.dram_tensor("wk", [D, DH], BF16, kind="ExternalInput").ap()
    d_wv = nc.dram_tensor("wv", [D, DH + 2], BF16, kind="ExternalInput").ap()
    d_bva = nc.dram_tensor("bva", [1, DH + 2], BF16, kind="ExternalInput").ap()
    d_wo = nc.dram_tensor("wo", [DH, D], F32R, kind="ExternalInput").ap()
    # ob: row 0 = ones, rows 1..12 = B mask (-1e9 where block r is not a
    # neighbor of q's block).  az: rows 0..11 = A block-indicator.
    d_ob = nc.dram_tensor("ob", [NB + 1, S], BF16, kind="ExternalInput").ap()
    d_az = nc.dram_tensor("az", [NB, S], BF16, kind="ExternalInput").ap()
    d_or2 = nc.dram_tensor("onesr2", [1, 2], F32R, kind="ExternalInput").ap()
    if with_bias:
        d_bq = nc.dram_tensor("bq", [1, DH], BF16, kind="ExternalInput").ap()
        d_bk = nc.dram_tensor("bk", [1, DH], BF16, kind="ExternalInput").ap()

    d_scores = nc.dram_tensor("scores", [S, M], F32, kind="ExternalOutput").ap()
    d_outp = nc.dram_tensor("outp", [S, D], F32, kind="ExternalOutput").ap()

    mm = nc.tensor.matmul

    with tile.TileContext(nc) as tc:
        with (
            tc.tile_pool(name="sb", bufs=1) as sb,
            tc.tile_pool(name="pp", bufs=4, space="PSUM") as pp,
            tc.tile_pool(name="pso", bufs=1, space="PSUM") as pso,
            tc.tile_pool(name="pctx", bufs=2, space="PSUM") as pctx,
        ):
            # ---- persistent SBUF tensors ----
            xq_sb = sb.tile([128, 2, S], BF16)
            xk_sb = sb.tile([128, 2, S], BF16)
            xv_sb = sb.tile([128, 2, S], BF16)
            wq_sb = sb.tile([128, 2, DH], BF16)
            wk_sb = sb.tile([128, 2, DH], BF16)
            wv_sb = sb.tile([128, 2, DH + 2], BF16)
            bva_sb = sb.tile([1, DH + 2], BF16)
            wo_sb = sb.tile([DH, D], F32R)
            ones_row = sb.tile([1, S], BF16)
            onesr2 = sb.tile([1, 2], F32R)
            qta = sb.tile([32 + NB, S], BF16)      # qT | B
            kta = sb.tile([32 + NB, S], BF16)      # kT | A
            vaug = sb.tile([128, NT, DH + 2], BF16)  # V | ones column
            et = sb.tile([128, NT, 2, 384], BF16)   # exp(S^T + mask)
            sout = sb.tile([64, NB, M], F32)       # gathered scores (per block)
            ctxu = sb.tile([DH + 1, S], F32R)      # ctx | rowsum row
            rs_sb = sb.tile([1, S], F32R)
            recip_col = sb.tile([128, 2 * NT], F32)
            outsb = sb.tile([128, NT, D], F32)
            if with_bias:
                bq_sb = sb.tile([1, DH], BF16)
                bk_sb = sb.tile([1, DH], BF16)

            # ---- input DMAs (weights first so matmuls can start early) ----
            nc.sync.dma_start(
                out=wq_sb[:, :, :], in_=d_wq.rearrange("(c p) d -> p c d", p=128)
            )
            for c in range(2):
                nc.sync.dma_start(
                    out=xq_sb[:, c, :], in_=d_xq[c * 128 : c * 128 + 128, :]
                )
            nc.sync.dma_start(
                out=wk_sb[:, :, :], in_=d_wk.rearrange("(c p) d -> p c d", p=128)
            )
            for c in range(2):
                nc.sync.dma_start(
                    out=xk_sb[:, c, :], in_=d_xk[c * 128 : c * 128 + 128, :]
                )
            nc.sync.dma_start(
                out=wv_sb[:, :, :], in_=d_wv.rearrange("(c p) d -> p c d", p=128)
            )
            for c in range(2):
                nc.sync.dma_start(
                    out=xv_sb[:, c, :], in_=d_xv[c * 128 : c * 128 + 128, :]
                )
            nc.sync.dma_start(out=ones_row[:, :], in_=d_ob[0:1, :])
            nc.sync.dma_start(out=bva_sb[:, :], in_=d_bva)
            nc.sync.dma_start(out=qta[32 : 32 + NB, :], in_=d_ob[1 : NB + 1, :])
            nc.sync.dma_start(out=kta[32 : 32 + NB, :], in_=d_az)
            nc.sync.dma_start(out=wo_sb[:, :], in_=d_wo)
            nc.sync.dma_start(out=onesr2[:, :], in_=d_or2)
            if with_bias:
                nc.sync.dma_start(out=bq_sb[:, :], in_=d_bq)
                nc.sync.dma_start(out=bk_sb[:, :], in_=d_bk)

            # ---- q/k projections: dst rows 0..31 = W.T @ X^T (+ bias) ----
            for pi, (wsb, xsb, dst) in enumerate(
                ((wq_sb, xq_sb, qta), (wk_sb, xk_sb, kta))
            ):
                for h2 in range(2):
                    ns = slice(h2 * 384, h2 * 384 + 384)
                    ps = pp.tile([32, 384], F32, tag="pp", name=f"psqk{pi}{h2}")
                    mm(ps[:, :], wsb[:, 0, :], xsb[:, 0, ns], start=True, stop=False)
                    mm(
                        ps[:, :],
                        wsb[:, 1, :],
                        xsb[:, 1, ns],
                        start=False,
                        stop=not with_bias,
                    )
                    if with_bias:
                        mm(
                            ps[:, :],
                            (bq_sb if pi == 0 else bk_sb)[:, :],
                            ones_row[0:1, ns],
                            start=False,
                            stop=True,
                        )
                    if pi == 0:
                        nc.vector.tensor_copy(dst[0:32, ns], ps[:, :])
                    else:
                        nc.scalar.copy(dst[0:32, ns], ps[:, :])

            # ---- V projection (+ ones column via the bias rank-1 matmul) ----
            for i in range(NT):
                ss = slice(i * 128, i * 128 + 128)
                pv = pp.tile([128, DH + 2], F32, tag="pp", name=f"psv{i}")
                mm(pv[:, :], xv_sb[:, 0, ss], wv_sb[:, 0, :], start=True, stop=False)
                mm(pv[:, :], xv_sb[:, 1, ss], wv_sb[:, 1, :], start=False, stop=False)
                mm(pv[:, :], ones_row[0:1, ss], bva_sb[:, :], start=False, stop=True)
                nc.vector.tensor_copy(vaug[:, i, :], pv[:, :])

            # ---- main interleaved loop over the 6 row-tiles ----
            ctx_ps = [
                pctx.tile([DH + 1, 384], F32, tag="pctx", name=f"psctx{h2}")
                for h2 in range(2)
            ]
            scores_view = d_scores.rearrange("(b p) m -> p b m", p=64)
            for i in range(NT):
                ts = slice(i * 128, i * 128 + 128)
                # S^T + mask (augmented contraction: rows 32.. add A x B)
                for h2 in range(2):
                    ns = slice(h2 * 384, h2 * 384 + 384)
                    st = pp.tile([128, 384], F32, tag="pp", name=f"psst{i}{h2}")
                    mm(
                        st[:, :],
                        kta[0 : 32 + NB, ts],
                        qta[0 : 32 + NB, ns],
                        start=True,
                        stop=True,
                    )
                    nc.scalar.activation(et[:, i, h2, :], st[:, :], EXP)
                # context + rowsum accumulation for this tile
                for h2 in range(2):
                    mm(
                        ctx_ps[h2][:, :],
                        vaug[:, i, 0 : DH + 1],
                        et[:, i, h2, :],
                        start=(i == 0),
                        stop=(i == NT - 1),
                    )
                # gathered scores for q-blocks 2i, 2i+1
                for b in (2 * i, 2 * i + 1):
                    ps = pso.tile([64, 2, 512], F32, tag="pso", name=f"psso{b}")
                    pieces, n64 = _pieces(nbrs[b])
                    for half, off, w, src in pieces:
                        mm(
                            ps[:, half, off : off + w],
                            qta[0:32, b * 64 : b * 64 + 64],
                            kta[0:32, src : src + w],
                            start=True,
                            stop=True,
                        )
                    w0 = min(n64, 384)
                    cp = nc.scalar.copy if b % 2 == 0 else nc.vector.tensor_copy
                    cp(sout[:, b, 0:w0], ps[:, 0, 0:w0])
                    if n64 > 384:
                        cp(sout[:, b, 384:n64], ps[:, 1, 0 : n64 - 384])
                    if n64 < M:
                        nc.vector.memset(sout[:, b, n64:M], NEG)
                    nc.gpsimd.dma_start(out=scores_view[:, b, :], in_=sout[:, b, :])

            # ---- rowsum -> per-partition scalars; normalize in out-proj ----
            for h2 in range(2):
                ns = slice(h2 * 384, h2 * 384 + 384)
                nc.scalar.copy(ctxu[:, ns], ctx_ps[h2][:, :])
                nc.scalar.copy(rs_sb[0:1, ns], ctx_ps[h2][DH : DH + 1, :])
            rs_col = pp.tile([128, 2 * NT], F32, tag="pp", name="psrscol")
            for i in range(NT):
                mm(
                    rs_col[:, 2 * i : 2 * i + 2],
                    rs_sb[0:1, i * 128 : i * 128 + 128],
                    onesr2[0:1, 0:2],
                    start=True,
                    stop=True,
                )
            nc.vector.reciprocal(recip_col[:, :], rs_col[:, :])

            # ---- output projection (partial: this head's slice of Wo) ----
            for i in range(NT):
                po = pp.tile([128, D], F32, tag="pp", name=f"pso{i}")
                mm(
                    po[:, :],
                    ctxu[0:DH, i * 128 : i * 128 + 128],
                    wo_sb[:, :],
                    start=True,
                    stop=True,
                )
                nc.vector.tensor_scalar_mul(
                    outsb[:, i, :], po[:, :], recip_col[:, 2 * i : 2 * i + 1]
                )
            nc.sync.dma_start(
                out=d_outp.rearrange("(i p) d -> p i d", p=128), in_=outsb[:, :, :]
            )

    nc.compile()
    return nc


def _host_prep(pattern):
    """Replicated constant tensors derived from the sparsity pattern."""
    nbrs = [list(t) for t in pattern]
    ones = np.ones((1, S), np.float32)
    Bm = np.full((NB, S), NEG, np.float32)
    for q in range(S):
        for r in nbrs[q // BS]:
            Bm[r, q] = 0.0
    Ap = np.zeros((NB, S), np.float32)
    for t in range(S):
        Ap[t // BS, t] = 1.0
    ob = np.concatenate([ones, Bm], axis=0).astype(NPBF)
    return ob, Ap.astype(NPBF)


_last_results = None  # stash for test harness introspection
_last_in_maps = None
_last_nc = None


def kernel(
    query,
    value,
    key_in,
    Wq,
    bq,
    Wk,
    bk,
    Wv,
    bv,
    Wo,
    bo,
    attn_indices,
    blocked_mask,
):
    query = np.asarray(query, np.float32)
    value = np.asarray(value, np.float32)
    key_in = np.asarray(key_in, np.float32)
    Wq = np.asarray(Wq, np.float32)
    Wk = np.asarray(Wk, np.float32)
    Wv = np.asarray(Wv, np.float32)
    Wo = np.asarray(Wo, np.float32)
    bq = np.asarray(bq, np.float32)
    bk = np.asarray(bk, np.float32)
    bv = np.asarray(bv, np.float32)
    bo = np.asarray(bo, np.float32)
    attn_indices = np.asarray(attn_indices, np.int32)
    blocked_mask_np = np.asarray(blocked_mask, np.float32)

    pattern = _derive_neighbors(attn_indices, blocked_mask_np)
    with_bias = bool(np.any(bq) or np.any(bk))
    key = (pattern, with_bias)
    if key not in _PROG_CACHE:
        _PROG_CACHE[key] = _build_program(pattern, with_bias)
    nc = _PROG_CACHE[key]

    ob, Ap = _host_prep(pattern)
    c = float(DH) ** -0.5
    xqt = np.ascontiguousarray(query.reshape(S, D).T).astype(NPBF)
    xkt = np.ascontiguousarray(key_in.reshape(S, D).T).astype(NPBF)
    xvt = np.ascontiguousarray(value.reshape(S, D).T).astype(NPBF)

    in_maps = []
    for h in range(NCORES):
        hs = slice(h * DH, (h + 1) * DH)
        wva = np.concatenate(
            [Wv[:, hs], np.zeros((D, 2), np.float32)], axis=1
        )  # col 32: ones column comes from the bias row below; col 33: pad
        bva = (
            np.concatenate([bv[hs], [1.0, 0.0]]).astype(np.float32).reshape(1, DH + 2)
        )
        im = {
            "xqt": xqt,
            "xkt": xkt,
            "xvt": xvt,
            "wq": np.ascontiguousarray(Wq[:, hs] * c).astype(NPBF),
            "wk": np.ascontiguousarray(Wk[:, hs]).astype(NPBF),
            "wv": np.ascontiguousarray(wva).astype(NPBF),
            "bva": bva.astype(NPBF),
            "wo": np.ascontiguousarray(Wo[hs, :]),
            "ob": ob,
            "az": Ap,
            "onesr2": np.ones((1, 2), np.float32),
        }
        if with_bias:
            im["bq"] = np.ascontiguousarray(bq[hs] * c).reshape(1, DH).astype(NPBF)
            im["bk"] = np.ascontiguousarray(bk[hs]).reshape(1, DH).astype(NPBF)
        in_maps.append(im)

    global _last_results, _last_in_maps, _last_nc
    _last_in_maps = in_maps
    _last_nc = nc
    res = run_bass_kernel_spmd(nc, in_maps, list(range(NCORES)))
    _last_results = res

    scores = np.stack([res.results[h]["scores"] for h in range(NCORES)])[None]
    out = np.zeros((S, D), np.float32)
    for h in range(NCORES):
        out += res.results[h]["outp"]
    out = (out + bo[None, :]).reshape(B, S, D)
    return out, scores, attn_indices
